# revision 45
# baseline (speedup 1.0000x reference)
"""DPQ joint classification loss on 8 Trainium2 NeuronCores.

reference math (B=4096, D=512, C=10000):
    soft_pred = soft_x @ weight.T ; hard_pred = hard_x @ weight.T
    loss = CE(soft_pred, t) + CE(hard_pred, t)
           + 0.5 * 0.5*(||soft_x - centers[t]||^2 + ||hard_x - centers[t]||^2) / B

Sharding: data-parallel over batch. Core i gets soft rows [i*512,(i+1)*512)
and the matching hard rows, stacked into X = [1024, 512]; weight/centers are
replicated. Each core returns one scalar:
    sum_rows( logsumexp(X @ W^T) - logit_at_target + 0.25*||X - centers[t]||^2 )
and the host computes loss = sum(cores) / B.

Per-core pipeline:
  - PE: fp8(e4m3) DoubleRow GEMM (256-deep contraction per matmul), rows on
    partitions (8 chunks of 128), classes streamed in 512-wide PSUM banks
    (4 banks per group), fp32 accumulation. Inputs are pre-scaled (x*16,
    w*64) to sit in the fp8 normal range; the activation un-scales by 1/1024
    before exp. Logit quantization noise (~0.02 std) is harmless: the
    dominant quant term and the target logit stay accurate.
  - ACT: exp straight out of PSUM with fused per-row accumulation
    (no max-subtraction: logits are ~N(0, 0.31), exp is safe in fp32).
  - GPSIMD: indirect-DMA row gathers weight[targets], centers[targets] (bf16).
  - DVE: target-logit (rowsum(x * w_gather)) and quantization
    (rowsum((x - c_gather)^2)) in bf16, final per-row combine in fp32.
  - PE again: cross-partition sum via ones-matmul; DMA scalar out.

DMA layout: xt8 and x are single consolidated tensors (one DMA each); the
weight stream is one DMA per class group. Aux traffic is bf16 to halve the
HBM bytes competing with the weight stream at startup.
"""

import json

import numpy as np

B_FULL = 4096
D = 512
C = 10000
N_CORES = 8
BS = B_FULL // N_CORES          # 512 rows per core per tensor
B = 2 * BS                      # 1024 stacked rows per core
P = 128
NB = B // P                     # 8 row chunks
NK = D // P                     # 4 contraction chunks
NKP = NK // 2                   # 2 DoubleRow contraction pairs (256 each)
GW = 2048                       # class-group width = 4 PSUM banks
GROUPS = [(0, 2048), (2048, 2048), (4096, 2048), (6144, 2048), (8192, 1808)]
PARAM = 0.5
X_SCALE = 16.0                  # fp8 pre-scales (undone in the exp activation)
W_SCALE = 64.0


def _patch_bir_bytes(b: bytes, max_waits: int = 1) -> bytes:
    """Adapt Tile-emitted BIR to this walrus build: it supports only one
    sync-wait per instruction (excess waits move to preceding NoOps) and
    rejects the EVENT_SEMAPHORE_RANGE_CLEAR raw-ISA encoding (replaced by
    per-semaphore write-0 EventSemaphore ops)."""
    d = json.loads(b)
    for f in d["functions"]:
        for blk in f["blocks"]:
            new_insts = []
            for ins in blk["instructions"]:
                if (
                    ins.get("opcode") == "ISA"
                    and ins.get("op_name") == "EVENT_SEMAPHORE_RANGE_CLEAR"
                ):
                    ad = ins.get("ant_dict") or {}
                    for sem_id in range(ad["range_first"], ad["range_last"] + 1):
                        new_insts.append({
                            "name": f"{ins['name']}_clr{sem_id}",
                            "opcode": "EventSemaphore",
                            "engine": ins["engine"],
                            "ins": [],
                            "outs": [],
                            "debug": ins.get("debug"),
                            "sync_info": {
                                "on_wait": [],
                                "on_update": [{
                                    "ant_name": f"semclr_{sem_id}",
                                    "id": sem_id,
                                    "sync_type": "semaphore",
                                    "update_mode": "sem-wr-imm",
                                    "update_value": 0,
                                }],
                            },
                        })
                    continue
                si = ins.get("sync_info")
                waits = (si or {}).get("on_wait") or []
                if len(waits) > max_waits:
                    extra, keep = waits[:-max_waits], waits[-max_waits:]
                    idx = 0
                    while extra:
                        chunk, extra = extra[:max_waits], extra[max_waits:]
                        new_insts.append({
                            "name": f"{ins['name']}_w{idx}",
                            "opcode": "NoOp",
                            "engine": ins["engine"],
                            "ins": [],
                            "outs": [],
                            "debug": ins.get("debug"),
                            "sync_info": {"on_wait": chunk, "on_update": []},
                        })
                        idx += 1
                    si["on_wait"] = keep
                new_insts.append(ins)
            blk["instructions"] = new_insts
    return json.dumps(d).encode()


def _build_bass():
    import concourse.bass as bass
    import concourse.tile as tile
    from concourse import mybir

    f32 = mybir.dt.float32
    bf16 = mybir.dt.bfloat16
    fp8 = mybir.dt.float8e4
    i32 = mybir.dt.int32
    AF = mybir.ActivationFunctionType
    OP = mybir.AluOpType
    DR = mybir.MatmulPerfMode.DoubleRow

    groups = GROUPS
    NG = len(groups)

    nc = bass.Bass()
    xt_d = nc.dram_tensor("xt", [P, NKP, 2, B], fp8, kind="ExternalInput")
    x_d = nc.dram_tensor("x", [P, NB, D], bf16, kind="ExternalInput")
    # one weight tensor per class group, contiguous per partition so the
    # HWDGE emits 8KB descriptors (2KB strided slices cap a queue at
    # ~120 GB/s; contiguous blocks run at engine rate)
    wt_ds = [
        nc.dram_tensor(f"wt{g}", [P, NKP, 2, cw], fp8, kind="ExternalInput")
        for g, (c0, cw) in enumerate(groups)
    ]
    w_d = nc.dram_tensor("w", [C, D], bf16, kind="ExternalInput")
    cen_d = nc.dram_tensor("cen", [C, D], bf16, kind="ExternalInput")
    tgt_d = nc.dram_tensor("tgt", [P, BS // P], i32, kind="ExternalInput")
    out_d = nc.dram_tensor("out", [1, 1], f32, kind="ExternalOutput")

    NC_ = BS // P               # 4 target chunks

    with tile.TileContext(nc) as tc:
        with (
            tc.tile_pool(name="persist", bufs=1) as persist,
            tc.tile_pool(name="wtp", bufs=2) as wtp,
            tc.tile_pool(name="scratch", bufs=3) as scratch,
        ):
            # ---- resident loads ----
            # The pipeline start needs xt + wt_g0 only (1.5 MB), both on the
            # sync HWDGE queue (contiguous blocks stream at ~400 GB/s there;
            # the scalar queue was measured drip-feeding its tail packets).
            # xt rides the gpsimd SWDGE queue so it transfers in parallel
            # with wt_g0 on the sync HWDGE queue (both gate the first unit).
            xt_sb = persist.tile([P, NKP, 2, B], fp8, name="xt")
            nc.gpsimd.dma_start(xt_sb[:, :, :, :], xt_d[:, :, :, :])
            wt_g0 = wtp.tile([P, NKP, 2, groups[0][1]], fp8, tag="wt", name="wt0")
            nc.sync.dma_start(wt_g0[:, :, :, :], wt_ds[0][:, :, :, :])

            # PE warmup: dummy DoubleRow matmuls bridge the DMA wait so the
            # PE p-state is fully ramped (3us continuous) when real work
            # starts with no idle gap in between.
            warm_l = persist.tile([P, 2, P], fp8, name="warm_l")
            warm_r = persist.tile([P, 2, 512], fp8, name="warm_r")
            nc.vector.memset(warm_l[:, :, :], 0.0)
            nc.vector.memset(warm_r[:, :, :], 0.0)
            with tc.tile_pool(name="psum_warm", bufs=1, space="PSUM") as pwarm:
                wps = pwarm.tile([P, 512], f32, name="wps")
                for i in range(9):
                    nc.tensor.matmul(
                        wps[:, :], lhsT=warm_l[:, :, :], rhs=warm_r[:, :, :],
                        start=True, stop=True, perf_mode=DR,
                    )

            # x/tgt feed only the (small) DVE aux path; gpsimd SWDGE keeps
            # them off the HWDGE queues. Dummy writes INTO x_sb/tgt_sb
            # (sourced from wt_g0) make their DMAs write-after-write
            # dependent on the group-0 weights, so their HBM traffic cannot
            # delay the pipeline start.
            g0w = groups[0][1]
            x_sb = persist.tile([P, NB, D], bf16, name="x")
            nc.gpsimd.tensor_copy(x_sb[:1, 0, :1], wt_g0[:1, 1, 1, g0w - 1:g0w])
            nc.gpsimd.dma_start(x_sb[:, :, :], x_d[:, :, :])
            tgt_sb = persist.tile([P, NC_], i32, name="tgt")
            nc.gpsimd.tensor_copy(tgt_sb[:1, :1], wt_g0[:1, 1, 1, g0w - 1:g0w])
            nc.gpsimd.dma_start(tgt_sb[:, :], tgt_d[:, :])

            # ---- gathers: weight[targets], centers[targets] (bf16) ----
            wg_sb, cg_sb = [], []
            for c in range(NC_):
                wg = persist.tile([P, D], bf16, tag=f"wg{c}", name=f"wg{c}")
                nc.gpsimd.indirect_dma_start(
                    out=wg[:, :], out_offset=None, in_=w_d[:, :],
                    in_offset=bass.IndirectOffsetOnAxis(ap=tgt_sb[:, c:c + 1], axis=0),
                )
                wg_sb.append(wg)
                cg = persist.tile([P, D], bf16, tag=f"cg{c}", name=f"cg{c}")
                nc.gpsimd.indirect_dma_start(
                    out=cg[:, :], out_offset=None, in_=cen_d[:, :],
                    in_offset=bass.IndirectOffsetOnAxis(ap=tgt_sb[:, c:c + 1], axis=0),
                )
                cg_sb.append(cg)

            # ---- small result tiles ----
            sums = persist.tile([P, NB, NG], f32, name="sums")
            se = persist.tile([P, NB], f32, name="se")
            lse = persist.tile([P, NB], f32, name="lse")
            tcol = persist.tile([P, NB], f32, name="tcol")
            qcol = persist.tile([P, NB], f32, name="qcol")
            ctr1 = persist.tile([P, NB], f32, name="ctr1")
            ctr2 = persist.tile([P, NB], f32, name="ctr2")
            rowtot = persist.tile([P, 1], f32, name="rowtot")
            ones = persist.tile([P, 1], f32, name="ones")
            nc.vector.memset(ones[:, :], 1.0)

            # ---- aux path on DVE: target logits + quantization (bf16) ----
            # (tensor_tensor_reduce lowers to a raw DVE ISA encoding this
            # walrus rejects, so use separate mul/sub + reduce ops)
            for b in range(NB):
                c = b % NC_
                pr = scratch.tile([P, D], bf16, tag="pr", name=f"pr{b}")
                nc.vector.tensor_mul(pr[:, :], x_sb[:, b, :], wg_sb[c][:, :])
                nc.vector.tensor_reduce(
                    out=tcol[:, b:b + 1], in_=pr[:, :],
                    axis=mybir.AxisListType.X, op=OP.add,
                )
                df = scratch.tile([P, D], bf16, tag="df", name=f"df{b}")
                nc.vector.tensor_sub(df[:, :], x_sb[:, b, :], cg_sb[c][:, :])
                sq = scratch.tile([P, D], bf16, tag="sq", name=f"sq{b}")
                nc.vector.tensor_mul(sq[:, :], df[:, :], df[:, :])
                nc.vector.tensor_reduce(
                    out=qcol[:, b:b + 1], in_=sq[:, :],
                    axis=mybir.AxisListType.X, op=OP.add,
                )

            # ---- main GEMM + exp/accumulate ----
            with tc.tile_pool(name="psum", bufs=2, space="PSUM") as psum_pool:
                for g, (c0, cw) in enumerate(groups):
                    if g == 0:
                        wt_g = wt_g0
                    else:
                        wt_g = wtp.tile([P, NKP, 2, cw], fp8, tag="wt", name=f"wt{g}")
                        nc.sync.dma_start(wt_g[:, :, :, :cw], wt_ds[g][:, :, :, :])
                    for b in range(NB):
                        ps = psum_pool.tile([P, cw], f32, tag="ps", name=f"ps{g}_{b}")
                        nbank = (cw + 511) // 512
                        for bank in range(nbank):
                            s0 = bank * 512
                            sw = min(512, cw - s0)
                            for kp in range(NKP):
                                nc.tensor.matmul(
                                    ps[:, s0:s0 + sw],
                                    lhsT=xt_sb[:, kp, :, b * P:(b + 1) * P],
                                    rhs=wt_g[:, kp, :, s0:s0 + sw],
                                    start=(kp == 0), stop=(kp == NKP - 1),
                                    perf_mode=DR,
                                )
                        es = scratch.tile([P, cw], bf16, tag="es", name=f"es{g}_{b}")
                        nc.scalar.activation(
                            es[:, :cw], ps[:, :cw], AF.Exp,
                            scale=1.0 / (X_SCALE * W_SCALE),
                            accum_out=sums[:, b, g:g + 1],
                        )

            # ---- logsumexp + per-row combine ----
            nc.vector.tensor_reduce(
                out=se[:, :], in_=sums[:, :, :],
                axis=mybir.AxisListType.X, op=OP.add,
            )
            nc.scalar.activation(lse[:, :], se[:, :], AF.Ln)
            nc.vector.tensor_sub(ctr1[:, :], lse[:, :], tcol[:, :])
            nc.vector.scalar_tensor_tensor(
                out=ctr2[:, :], in0=qcol[:, :], scalar=0.25, in1=ctr1[:, :],
                op0=OP.mult, op1=OP.add,
            )
            nc.vector.tensor_reduce(
                out=rowtot[:, :], in_=ctr2[:, :],
                axis=mybir.AxisListType.X, op=OP.add,
            )

            # ---- cross-partition sum via ones-matmul, write scalar ----
            with tc.tile_pool(name="psum2", bufs=1, space="PSUM") as pp2:
                tot_ps = pp2.tile([1, 1], f32, name="tot_ps")
                nc.tensor.matmul(
                    tot_ps[:, :], lhsT=rowtot[:, :], rhs=ones[:, :],
                    start=True, stop=True,
                )
                tot_sb = persist.tile([1, 1], f32, name="tot_sb")
                nc.vector.tensor_copy(tot_sb[:, :], tot_ps[:, :])
                nc.sync.dma_start(out_d[:, :], tot_sb[:, :])

    orig_to_json = nc.to_json_bytes
    nc.to_json_bytes = lambda: _patch_bir_bytes(orig_to_json())
    return nc


_NC = None


def _get_nc():
    global _NC
    if _NC is None:
        _NC = _build_bass()
    return _NC


def _make_in_maps(soft_x, hard_x, targets, centers, weight):
    import ml_dtypes

    soft_x = np.asarray(soft_x, np.float32)
    hard_x = np.asarray(hard_x, np.float32)
    targets = np.asarray(targets)
    weight = np.asarray(weight, np.float32)
    centers = np.asarray(centers, np.float32)
    fp8 = ml_dtypes.float8_e4m3
    bf16 = ml_dtypes.bfloat16

    # [D, C] -> [P, NKP, 2, C]: contraction split into DoubleRow pairs,
    # partition dim leading; then one contiguous array per class group.
    wt8 = np.ascontiguousarray(
        (weight.T * W_SCALE).reshape(NKP, 2, P, C).transpose(2, 0, 1, 3)
    ).astype(fp8)
    wt8_groups = {
        f"wt{g}": np.ascontiguousarray(wt8[:, :, :, c0:c0 + cw])
        for g, (c0, cw) in enumerate(GROUPS)
    }
    w_bf = np.ascontiguousarray(weight).astype(bf16)
    cen_bf = np.ascontiguousarray(centers).astype(bf16)

    in_maps = []
    for i in range(N_CORES):
        sl = slice(i * BS, (i + 1) * BS)
        X = np.concatenate([soft_x[sl], hard_x[sl]], axis=0)
        xt8 = np.ascontiguousarray(
            (X.T * X_SCALE).reshape(NKP, 2, P, B).transpose(2, 0, 1, 3)
        ).astype(fp8)
        # [B, D] -> [P, NB, D] (row chunk b on partitions)
        xb = np.ascontiguousarray(
            X.reshape(NB, P, D).transpose(1, 0, 2)
        ).astype(bf16)
        tg = np.ascontiguousarray(
            targets[sl].astype(np.int32).reshape(BS // P, P).T
        )
        m = {"xt": xt8, "x": xb, "w": w_bf, "cen": cen_bf, "tgt": tg}
        m.update(wt8_groups)
        in_maps.append(m)
    return in_maps


def _run(inputs, trace=False):
    from concourse.bass_utils import run_bass_kernel_spmd

    nc = _get_nc()
    in_maps = _make_in_maps(**inputs)
    res = run_bass_kernel_spmd(
        nc, in_maps, core_ids=list(range(N_CORES)), trace=trace
    )
    total = sum(float(r["out"][0, 0]) for r in res.results)
    return np.float32(total / B_FULL), res


def kernel(soft_x, hard_x, targets, centers, weight):
    loss, _ = _run(
        dict(soft_x=soft_x, hard_x=hard_x, targets=targets,
             centers=centers, weight=weight)
    )
    return loss


# revision 46
# speedup vs baseline: 1.0431x; 1.0431x over previous
"""DPQ joint classification loss on 8 Trainium2 NeuronCores.

reference math (B=4096, D=512, C=10000):
    soft_pred = soft_x @ weight.T ; hard_pred = hard_x @ weight.T
    loss = CE(soft_pred, t) + CE(hard_pred, t)
           + 0.5 * 0.5*(||soft_x - centers[t]||^2 + ||hard_x - centers[t]||^2) / B

Sharding: data-parallel over batch. Core i gets soft rows [i*512,(i+1)*512)
and the matching hard rows, stacked into X = [1024, 512]; weight/centers are
replicated. Each core returns one scalar:
    sum_rows( logsumexp(X @ W^T) - logit_at_target + 0.25*||X - centers[t]||^2 )
and the host computes loss = sum(cores) / B.

Per-core pipeline:
  - PE: fp8(e4m3) DoubleRow GEMM (256-deep contraction per matmul), rows on
    partitions (8 chunks of 128), classes streamed in 512-wide PSUM banks
    (4 banks per group), fp32 accumulation. Inputs are pre-scaled (x*16,
    w*64) to sit in the fp8 normal range; the activation un-scales by 1/1024
    before exp. Logit quantization noise (~0.02 std) is harmless: the
    dominant quant term and the target logit stay accurate.
  - ACT: exp straight out of PSUM with fused per-row accumulation
    (no max-subtraction: logits are ~N(0, 0.31), exp is safe in fp32).
  - GPSIMD: indirect-DMA row gathers weight[targets], centers[targets] (bf16).
  - DVE: target-logit (rowsum(x * w_gather)) and quantization
    (rowsum((x - c_gather)^2)) in bf16, final per-row combine in fp32.
  - PE again: cross-partition sum via ones-matmul; DMA scalar out.

DMA layout: xt8 and x are single consolidated tensors (one DMA each); the
weight stream is one DMA per class group. Aux traffic is bf16 to halve the
HBM bytes competing with the weight stream at startup.
"""

import json

import numpy as np

B_FULL = 4096
D = 512
C = 10000
N_CORES = 8
BS = B_FULL // N_CORES          # 512 rows per core per tensor
B = 2 * BS                      # 1024 stacked rows per core
P = 128
NB = B // P                     # 8 row chunks
NK = D // P                     # 4 contraction chunks
NKP = NK // 2                   # 2 DoubleRow contraction pairs (256 each)
GW = 2048                       # class-group width = 4 PSUM banks
GROUPS = [(0, 2048), (2048, 2048), (4096, 2048), (6144, 2048), (8192, 1808)]
PARAM = 0.5
X_SCALE = 16.0                  # fp8 pre-scales (undone in the exp activation)
W_SCALE = 64.0


def _patch_bir_bytes(b: bytes, max_waits: int = 1) -> bytes:
    """Adapt Tile-emitted BIR to this walrus build: it supports only one
    sync-wait per instruction (excess waits move to preceding NoOps) and
    rejects the EVENT_SEMAPHORE_RANGE_CLEAR raw-ISA encoding (replaced by
    per-semaphore write-0 EventSemaphore ops)."""
    d = json.loads(b)
    for f in d["functions"]:
        for blk in f["blocks"]:
            new_insts = []
            for ins in blk["instructions"]:
                if (
                    ins.get("opcode") == "ISA"
                    and ins.get("op_name") == "EVENT_SEMAPHORE_RANGE_CLEAR"
                ):
                    ad = ins.get("ant_dict") or {}
                    for sem_id in range(ad["range_first"], ad["range_last"] + 1):
                        new_insts.append({
                            "name": f"{ins['name']}_clr{sem_id}",
                            "opcode": "EventSemaphore",
                            "engine": ins["engine"],
                            "ins": [],
                            "outs": [],
                            "debug": ins.get("debug"),
                            "sync_info": {
                                "on_wait": [],
                                "on_update": [{
                                    "ant_name": f"semclr_{sem_id}",
                                    "id": sem_id,
                                    "sync_type": "semaphore",
                                    "update_mode": "sem-wr-imm",
                                    "update_value": 0,
                                }],
                            },
                        })
                    continue
                si = ins.get("sync_info")
                waits = (si or {}).get("on_wait") or []
                if len(waits) > max_waits:
                    extra, keep = waits[:-max_waits], waits[-max_waits:]
                    idx = 0
                    while extra:
                        chunk, extra = extra[:max_waits], extra[max_waits:]
                        new_insts.append({
                            "name": f"{ins['name']}_w{idx}",
                            "opcode": "NoOp",
                            "engine": ins["engine"],
                            "ins": [],
                            "outs": [],
                            "debug": ins.get("debug"),
                            "sync_info": {"on_wait": chunk, "on_update": []},
                        })
                        idx += 1
                    si["on_wait"] = keep
                new_insts.append(ins)
            blk["instructions"] = new_insts
    return json.dumps(d).encode()


def _build_bass():
    import concourse.bass as bass
    import concourse.tile as tile
    from concourse import mybir

    f32 = mybir.dt.float32
    bf16 = mybir.dt.bfloat16
    fp8 = mybir.dt.float8e4
    i32 = mybir.dt.int32
    AF = mybir.ActivationFunctionType
    OP = mybir.AluOpType
    DR = mybir.MatmulPerfMode.DoubleRow

    groups = GROUPS
    NG = len(groups)

    nc = bass.Bass()
    xt_d = nc.dram_tensor("xt", [P, NKP, 2, B], fp8, kind="ExternalInput")
    x_d = nc.dram_tensor("x", [P, NB, D], bf16, kind="ExternalInput")
    # one weight tensor per class group, contiguous per partition so the
    # HWDGE emits 8KB descriptors (2KB strided slices cap a queue at
    # ~120 GB/s; contiguous blocks run at engine rate)
    wt_ds = [
        nc.dram_tensor(f"wt{g}", [P, NKP, 2, cw], fp8, kind="ExternalInput")
        for g, (c0, cw) in enumerate(groups)
    ]
    w_d = nc.dram_tensor("w", [C, D], bf16, kind="ExternalInput")
    cen_d = nc.dram_tensor("cen", [C, D], bf16, kind="ExternalInput")
    tgt_d = nc.dram_tensor("tgt", [P, BS // P], i32, kind="ExternalInput")
    out_d = nc.dram_tensor("out", [1, 1], f32, kind="ExternalOutput")

    NC_ = BS // P               # 4 target chunks

    with tile.TileContext(nc) as tc:
        with (
            tc.tile_pool(name="persist", bufs=1) as persist,
            tc.tile_pool(name="wtp", bufs=2) as wtp,
            tc.tile_pool(name="scratch", bufs=3) as scratch,
        ):
            # ---- resident loads ----
            # The pipeline start needs xt + wt_g0 only (1.5 MB), both on the
            # sync HWDGE queue (contiguous blocks stream at ~400 GB/s there;
            # the scalar queue was measured drip-feeding its tail packets).
            xt_sb = persist.tile([P, NKP, 2, B], fp8, name="xt")
            nc.sync.dma_start(xt_sb[:, :, :, :], xt_d[:, :, :, :])
            wt_g0 = wtp.tile([P, NKP, 2, groups[0][1]], fp8, tag="wt", name="wt0")
            nc.sync.dma_start(wt_g0[:, :, :, :], wt_ds[0][:, :, :, :])

            # PE warmup: dummy DoubleRow matmuls bridge the DMA wait so the
            # PE p-state is fully ramped (3us continuous) when real work
            # starts with no idle gap in between.
            warm_l = persist.tile([P, 2, P], fp8, name="warm_l")
            warm_r = persist.tile([P, 2, 512], fp8, name="warm_r")
            nc.vector.memset(warm_l[:, :, :], 0.0)
            nc.vector.memset(warm_r[:, :, :], 0.0)
            with tc.tile_pool(name="psum_warm", bufs=1, space="PSUM") as pwarm:
                wps = pwarm.tile([P, 512], f32, name="wps")
                for i in range(9):
                    nc.tensor.matmul(
                        wps[:, :], lhsT=warm_l[:, :, :], rhs=warm_r[:, :, :],
                        start=True, stop=True, perf_mode=DR,
                    )

            # x/tgt feed only the (small) DVE aux path; gpsimd SWDGE keeps
            # them off the HWDGE queues. Dummy writes INTO x_sb/tgt_sb
            # (sourced from wt_g0) make their DMAs write-after-write
            # dependent on the group-0 weights, so their HBM traffic cannot
            # delay the pipeline start.
            g0w = groups[0][1]
            x_sb = persist.tile([P, NB, D], bf16, name="x")
            nc.gpsimd.tensor_copy(x_sb[:1, 0, :1], wt_g0[:1, 1, 1, g0w - 1:g0w])
            nc.gpsimd.dma_start(x_sb[:, :, :], x_d[:, :, :])
            tgt_sb = persist.tile([P, NC_], i32, name="tgt")
            nc.gpsimd.tensor_copy(tgt_sb[:1, :1], wt_g0[:1, 1, 1, g0w - 1:g0w])
            nc.gpsimd.dma_start(tgt_sb[:, :], tgt_d[:, :])

            # ---- gathers: weight[targets], centers[targets] (bf16) ----
            wg_sb, cg_sb = [], []
            for c in range(NC_):
                wg = persist.tile([P, D], bf16, tag=f"wg{c}", name=f"wg{c}")
                nc.gpsimd.indirect_dma_start(
                    out=wg[:, :], out_offset=None, in_=w_d[:, :],
                    in_offset=bass.IndirectOffsetOnAxis(ap=tgt_sb[:, c:c + 1], axis=0),
                )
                wg_sb.append(wg)
                cg = persist.tile([P, D], bf16, tag=f"cg{c}", name=f"cg{c}")
                nc.gpsimd.indirect_dma_start(
                    out=cg[:, :], out_offset=None, in_=cen_d[:, :],
                    in_offset=bass.IndirectOffsetOnAxis(ap=tgt_sb[:, c:c + 1], axis=0),
                )
                cg_sb.append(cg)

            # ---- small result tiles ----
            sums = persist.tile([P, NB, NG], f32, name="sums")
            se = persist.tile([P, NB], f32, name="se")
            lse = persist.tile([P, NB], f32, name="lse")
            tcol = persist.tile([P, NB], f32, name="tcol")
            qcol = persist.tile([P, NB], f32, name="qcol")
            ctr1 = persist.tile([P, NB], f32, name="ctr1")
            ctr2 = persist.tile([P, NB], f32, name="ctr2")
            rowtot = persist.tile([P, 1], f32, name="rowtot")
            ones = persist.tile([P, 1], f32, name="ones")
            nc.vector.memset(ones[:, :], 1.0)

            # ---- aux path on DVE: target logits + quantization (bf16) ----
            # (tensor_tensor_reduce lowers to a raw DVE ISA encoding this
            # walrus rejects, so use separate mul/sub + reduce ops)
            for b in range(NB):
                c = b % NC_
                pr = scratch.tile([P, D], bf16, tag="pr", name=f"pr{b}")
                nc.vector.tensor_mul(pr[:, :], x_sb[:, b, :], wg_sb[c][:, :])
                nc.vector.tensor_reduce(
                    out=tcol[:, b:b + 1], in_=pr[:, :],
                    axis=mybir.AxisListType.X, op=OP.add,
                )
                df = scratch.tile([P, D], bf16, tag="df", name=f"df{b}")
                nc.vector.tensor_sub(df[:, :], x_sb[:, b, :], cg_sb[c][:, :])
                sq = scratch.tile([P, D], bf16, tag="sq", name=f"sq{b}")
                nc.vector.tensor_mul(sq[:, :], df[:, :], df[:, :])
                nc.vector.tensor_reduce(
                    out=qcol[:, b:b + 1], in_=sq[:, :],
                    axis=mybir.AxisListType.X, op=OP.add,
                )

            # ---- main GEMM + exp/accumulate ----
            with tc.tile_pool(name="psum", bufs=2, space="PSUM") as psum_pool:
                for g, (c0, cw) in enumerate(groups):
                    if g == 0:
                        wt_g = wt_g0
                    else:
                        wt_g = wtp.tile([P, NKP, 2, cw], fp8, tag="wt", name=f"wt{g}")
                        nc.sync.dma_start(wt_g[:, :, :, :cw], wt_ds[g][:, :, :, :])
                    for b in range(NB):
                        ps = psum_pool.tile([P, cw], f32, tag="ps", name=f"ps{g}_{b}")
                        nbank = (cw + 511) // 512
                        for bank in range(nbank):
                            s0 = bank * 512
                            sw = min(512, cw - s0)
                            for kp in range(NKP):
                                nc.tensor.matmul(
                                    ps[:, s0:s0 + sw],
                                    lhsT=xt_sb[:, kp, :, b * P:(b + 1) * P],
                                    rhs=wt_g[:, kp, :, s0:s0 + sw],
                                    start=(kp == 0), stop=(kp == NKP - 1),
                                    perf_mode=DR,
                                )
                        es = scratch.tile([P, cw], bf16, tag="es", name=f"es{g}_{b}")
                        nc.scalar.activation(
                            es[:, :cw], ps[:, :cw], AF.Exp,
                            scale=1.0 / (X_SCALE * W_SCALE),
                            accum_out=sums[:, b, g:g + 1],
                        )

            # ---- logsumexp + per-row combine ----
            nc.vector.tensor_reduce(
                out=se[:, :], in_=sums[:, :, :],
                axis=mybir.AxisListType.X, op=OP.add,
            )
            nc.scalar.activation(lse[:, :], se[:, :], AF.Ln)
            nc.vector.tensor_sub(ctr1[:, :], lse[:, :], tcol[:, :])
            nc.vector.scalar_tensor_tensor(
                out=ctr2[:, :], in0=qcol[:, :], scalar=0.25, in1=ctr1[:, :],
                op0=OP.mult, op1=OP.add,
            )
            nc.vector.tensor_reduce(
                out=rowtot[:, :], in_=ctr2[:, :],
                axis=mybir.AxisListType.X, op=OP.add,
            )

            # ---- cross-partition sum via ones-matmul, write scalar ----
            with tc.tile_pool(name="psum2", bufs=1, space="PSUM") as pp2:
                tot_ps = pp2.tile([1, 1], f32, name="tot_ps")
                nc.tensor.matmul(
                    tot_ps[:, :], lhsT=rowtot[:, :], rhs=ones[:, :],
                    start=True, stop=True,
                )
                tot_sb = persist.tile([1, 1], f32, name="tot_sb")
                nc.vector.tensor_copy(tot_sb[:, :], tot_ps[:, :])
                nc.sync.dma_start(out_d[:, :], tot_sb[:, :])

    orig_to_json = nc.to_json_bytes
    nc.to_json_bytes = lambda: _patch_bir_bytes(orig_to_json())
    return nc


_NC = None


def _get_nc():
    global _NC
    if _NC is None:
        _NC = _build_bass()
    return _NC


def _make_in_maps(soft_x, hard_x, targets, centers, weight):
    import ml_dtypes

    soft_x = np.asarray(soft_x, np.float32)
    hard_x = np.asarray(hard_x, np.float32)
    targets = np.asarray(targets)
    weight = np.asarray(weight, np.float32)
    centers = np.asarray(centers, np.float32)
    fp8 = ml_dtypes.float8_e4m3
    bf16 = ml_dtypes.bfloat16

    # [D, C] -> [P, NKP, 2, C]: contraction split into DoubleRow pairs,
    # partition dim leading; then one contiguous array per class group.
    wt8 = np.ascontiguousarray(
        (weight.T * W_SCALE).reshape(NKP, 2, P, C).transpose(2, 0, 1, 3)
    ).astype(fp8)
    wt8_groups = {
        f"wt{g}": np.ascontiguousarray(wt8[:, :, :, c0:c0 + cw])
        for g, (c0, cw) in enumerate(GROUPS)
    }
    w_bf = np.ascontiguousarray(weight).astype(bf16)
    cen_bf = np.ascontiguousarray(centers).astype(bf16)

    in_maps = []
    for i in range(N_CORES):
        sl = slice(i * BS, (i + 1) * BS)
        X = np.concatenate([soft_x[sl], hard_x[sl]], axis=0)
        xt8 = np.ascontiguousarray(
            (X.T * X_SCALE).reshape(NKP, 2, P, B).transpose(2, 0, 1, 3)
        ).astype(fp8)
        # [B, D] -> [P, NB, D] (row chunk b on partitions)
        xb = np.ascontiguousarray(
            X.reshape(NB, P, D).transpose(1, 0, 2)
        ).astype(bf16)
        tg = np.ascontiguousarray(
            targets[sl].astype(np.int32).reshape(BS // P, P).T
        )
        m = {"xt": xt8, "x": xb, "w": w_bf, "cen": cen_bf, "tgt": tg}
        m.update(wt8_groups)
        in_maps.append(m)
    return in_maps


def _run(inputs, trace=False):
    from concourse.bass_utils import run_bass_kernel_spmd

    nc = _get_nc()
    in_maps = _make_in_maps(**inputs)
    res = run_bass_kernel_spmd(
        nc, in_maps, core_ids=list(range(N_CORES)), trace=trace
    )
    total = sum(float(r["out"][0, 0]) for r in res.results)
    return np.float32(total / B_FULL), res


def kernel(soft_x, hard_x, targets, centers, weight):
    loss, _ = _run(
        dict(soft_x=soft_x, hard_x=hard_x, targets=targets,
             centers=centers, weight=weight)
    )
    return loss
